# revision 1
# baseline (speedup 1.0000x reference)
import numpy as np

_CACHE = {}

N_CORES = 8
TOK = 16384
TOK_PER = TOK // N_CORES  # 2048 tokens per core
DIM = 2048
NE = 64
TOPK = 8
KC = 128            # contraction chunk (partition dim)
NK = DIM // KC      # 16 chunks
NT = 512            # token tile = one f32 PSUM bank
NJ = TOK_PER // NT  # 4 token tiles


def _build():
    import concourse.bass as bass
    import concourse.tile as tile
    from concourse import bacc, mybir

    nc = bacc.Bacc(
        "TRN2",
        target_bir_lowering=False,
        debug=False,
        enable_asserts=False,
        num_devices=N_CORES,
    )
    xT = nc.dram_tensor("xT", (DIM, TOK_PER), mybir.dt.float32, kind="ExternalInput").ap()
    wT = nc.dram_tensor("WT", (DIM, NE), mybir.dt.float32, kind="ExternalInput").ap()
    out = nc.dram_tensor("logitsT", (NE, TOK_PER), mybir.dt.float32, kind="ExternalOutput").ap()

    with tile.TileContext(nc) as tc:
        with (
            tc.tile_pool(name="xpool", bufs=NK) as xpool,
            tc.tile_pool(name="wpool", bufs=NK) as wpool,
            tc.tile_pool(name="opool", bufs=NJ) as opool,
            tc.tile_pool(name="psum", bufs=NJ, space=bass.MemorySpace.PSUM) as psum,
        ):
            xts, wts = [], []
            for k in range(NK):
                xt = xpool.tile([KC, TOK_PER], mybir.dt.float32)
                nc.gpsimd.dma_start(xt[:], xT[k * KC:(k + 1) * KC, :])
                wt = wpool.tile([KC, NE], mybir.dt.float32)
                nc.gpsimd.dma_start(wt[:], wT[k * KC:(k + 1) * KC, :])
                xts.append(xt)
                wts.append(wt)
            for j in range(NJ):
                acc = psum.tile([NE, NT], mybir.dt.float32)
                for k in range(NK):
                    nc.tensor.matmul(
                        acc[:],
                        wts[k][:],
                        xts[k][:, j * NT:(j + 1) * NT],
                        start=(k == 0),
                        stop=(k == NK - 1),
                    )
                ot = opool.tile([NE, NT], mybir.dt.float32)
                nc.vector.tensor_copy(ot[:], acc[:])
                nc.gpsimd.dma_start(out[:, j * NT:(j + 1) * NT], ot[:])
    nc.compile()
    return nc


def kernel(x, W):
    from concourse import bass_utils

    x = np.asarray(x, dtype=np.float32)
    W = np.asarray(W, dtype=np.float32)
    if "nc" not in _CACHE:
        _CACHE["nc"] = _build()
    nc = _CACHE["nc"]

    WT = np.ascontiguousarray(W.T)
    in_maps = []
    for i in range(N_CORES):
        xs = x[i * TOK_PER:(i + 1) * TOK_PER]
        in_maps.append({"xT": np.ascontiguousarray(xs.T), "WT": WT})
    res = bass_utils.run_bass_kernel_spmd(nc, in_maps, list(range(N_CORES)))
    logits = np.concatenate(
        [np.asarray(r["logitsT"]).T for r in res.results], axis=0
    ).astype(np.float32)

    m = logits.max(axis=-1, keepdims=True)
    e = np.exp(logits - m)
    scores = e / e.sum(axis=-1, keepdims=True)
    idx = np.argsort(-scores, axis=-1, kind="stable")[:, :TOPK].astype(np.int32)
    w = np.take_along_axis(scores, idx, axis=-1).astype(np.float32)
    return w, idx



# revision 5
# speedup vs baseline: 1.3411x; 1.3411x over previous
import numpy as np

_CACHE = {}

N_CORES = 8
TOK = 16384
TOK_PER = TOK // N_CORES  # 2048 tokens per core
DIM = 2048
NE = 64
TOPK = 8
KC = 128            # contraction chunk (partition dim)
NK = DIM // KC      # 16 chunks
NT = 512            # token tile = one f32 PSUM bank
NJ = TOK_PER // NT  # 4 token tiles
N_WARM = 12         # PE warm-up matmuls before real data arrives


def _build():
    import concourse.bass as bass
    import concourse.tile as tile
    from concourse import bacc, mybir

    nc = bacc.Bacc(
        "TRN2",
        target_bir_lowering=False,
        debug=False,
        enable_asserts=False,
        num_devices=N_CORES,
    )
    # x2: row d = [xh_d (TOK_PER f16) | xl_d (TOK_PER f16)] -> contiguous 1MB
    # per 128-row chunk
    x2 = nc.dram_tensor("x2", (DIM, 2 * TOK_PER), mybir.dt.float16, kind="ExternalInput").ap()
    # wc: partition-major packed [Wh | Wl]: wc[p, k*128 + c] = Wcat[k*128 + p, c]
    wc = nc.dram_tensor("wc", (KC, NK * 2 * NE), mybir.dt.float16, kind="ExternalInput").ap()
    # out rows 0-63 = sum_k x*Wh contribution, rows 64-127 = x*Wl; host folds
    out = nc.dram_tensor("o2", (2 * NE, TOK_PER), mybir.dt.float32, kind="ExternalOutput").ap()

    f16 = mybir.dt.float16
    f32 = mybir.dt.float32

    with tile.TileContext(nc) as tc:
        with (
            tc.tile_pool(name="warm", bufs=1) as warmpool,
            tc.tile_pool(name="wpool", bufs=1) as wpool,
            tc.tile_pool(name="xpool", bufs=NK) as xpool,
            tc.tile_pool(name="opool", bufs=NJ) as opool,
            tc.tile_pool(name="psum", bufs=1, space=bass.MemorySpace.PSUM) as psum,
            tc.tile_pool(name="psumw", bufs=1, space=bass.MemorySpace.PSUM) as psumw,
        ):
            # --- PE warm-up: keep TensorE busy from kernel start so HAM
            # unthrottles to 2.4GHz before the real matmuls begin.
            wsrc = warmpool.tile([KC, 2 * NE], f16)
            wmov = warmpool.tile([KC, NT], f16)
            nc.gpsimd.memset(wsrc[:], 0.0)
            nc.gpsimd.memset(wmov[:], 0.0)
            wacc = psumw.tile([2 * NE, NT], f32)
            for _ in range(N_WARM):
                nc.tensor.matmul(wacc[:], wsrc[:], wmov[:], start=True, stop=True)

            # --- input DMAs (HWDGE via sync engine): W first, then x chunks
            wt = wpool.tile([KC, NK * 2 * NE], f16)
            nc.sync.dma_start(wt[:], wc[:, :])
            xts = []
            for k in range(NK):
                xt = xpool.tile([KC, 2 * TOK_PER], f16)
                nc.sync.dma_start(xt[:], x2[k * KC:(k + 1) * KC, :])
                xts.append(xt)

            # --- matmuls: stationary = [Wh_k | Wl_k] (128 cols); for each k
            # stream hi then lo moving tiles; PSUM rows 0-63 accumulate the
            # Wh product, rows 64-127 the Wl product.
            accs = [
                psum.tile([2 * NE, NT], f32, name=f"acc{j}", tag=f"acc{j}")
                for j in range(NJ)
            ]
            for k in range(NK):
                wk = wt[:, k * 2 * NE:(k + 1) * 2 * NE]
                for j in range(NJ):
                    nc.tensor.matmul(
                        accs[j][:],
                        wk,
                        xts[k][:, j * NT:(j + 1) * NT],
                        start=(k == 0),
                        stop=False,
                    )
                    nc.tensor.matmul(
                        accs[j][:],
                        wk,
                        xts[k][:, TOK_PER + j * NT:TOK_PER + (j + 1) * NT],
                        start=False,
                        stop=(k == NK - 1),
                    )
            for j in range(NJ):
                ot = opool.tile([2 * NE, NT], f32)
                nc.vector.tensor_copy(ot[:], accs[j][:])
                nc.scalar.dma_start(out[:, j * NT:(j + 1) * NT], ot[:])
    nc.compile()
    return nc


def _prepare_in_maps(x, W):
    x = np.asarray(x, dtype=np.float32)
    W = np.asarray(W, dtype=np.float32)

    # W: transpose to (DIM, NE), fp16 hi/lo split, pack [Wh | Wl] along cols,
    # then partition-major relayout wc[p, k*128 + c] = Wcat[k*128 + p, c]
    WT = np.ascontiguousarray(W.T)                       # (DIM, NE)
    Wh = WT.astype(np.float16)
    Wl = (WT - Wh.astype(np.float32)).astype(np.float16)
    Wcat = np.concatenate([Wh, Wl], axis=1)              # (DIM, 128)
    wc = np.ascontiguousarray(
        Wcat.reshape(NK, KC, 2 * NE).transpose(1, 0, 2).reshape(KC, NK * 2 * NE)
    )

    in_maps = []
    for i in range(N_CORES):
        xsT = x[i * TOK_PER:(i + 1) * TOK_PER].T         # (DIM, TOK_PER) view
        xh = xsT.astype(np.float16)
        xl = (xsT - xh.astype(np.float32)).astype(np.float16)
        x2 = np.ascontiguousarray(np.concatenate([xh, xl], axis=1))
        in_maps.append({"x2": x2, "wc": wc})
    return in_maps


def kernel(x, W):
    from concourse import bass_utils

    if "nc" not in _CACHE:
        _CACHE["nc"] = _build()
    nc = _CACHE["nc"]

    in_maps = _prepare_in_maps(x, W)
    res = bass_utils.run_bass_kernel_spmd(nc, in_maps, list(range(N_CORES)))
    logits = np.concatenate(
        [np.asarray(r["o2"][:NE]) + np.asarray(r["o2"][NE:]) for r in res.results],
        axis=1,
    ).T.astype(np.float32)                               # (TOK, NE)

    m = logits.max(axis=-1, keepdims=True)
    e = np.exp(logits - m)
    scores = e / e.sum(axis=-1, keepdims=True)
    idx = np.argsort(-scores, axis=-1, kind="stable")[:, :TOPK].astype(np.int32)
    w = np.take_along_axis(scores, idx, axis=-1).astype(np.float32)
    return w, idx
